# revision 19
# baseline (speedup 1.0000x reference)
"""Trainium2 Bass kernel for MultiHeadLatentAttention (MLA), 8-core SPMD.

Sharding: core c = (batch b=c//4, head-group g=c%4 of 4 heads).
Each core computes the full latent down-projections for its batch
(replicated across the 4 cores of that batch), head-sharded
up-projections + RoPE + causal attention, and a partial o-projection
(its heads' rows of Wo). The host sums the 4 partial outputs per batch.

Shapes (fixed): B=2, S=2048, H=2048, L=256, nh=16, hd=128, rd=64.

Layouts on device (all transposed, feature-on-partitions):
  kv_dT, q_dT [L=256 -> 2x128, s]   k_rT [4 heads * 64 -> 2x128, s]
  qT_h, kT_h  [128 (64 content + 64 rope), s]  per head
  v_all       [128 (s%128), s//128, 4 heads * 128]  (natural v)
  yT_all      [128 (hd), 4 heads, s]

Attention is computed in scores-transposed orientation S^T[k, q] so the
probabilities feed the AV matmul directly (lhsT = v block, rhs = expS).
Softmax skips the max-subtraction (scores are provably tiny here:
|s| < ~2) and gets the denominator from an all-ones matmul over the
accumulated exp blocks, which also broadcasts it across partitions.
"""

import sys
import os

sys.path.insert(0, "/opt/trn_rl_repo")

import numpy as np

B = 2
S = 2048
H = 2048
L = 256          # latent dim (2 chunks of 128)
NH = 16          # total heads
HPC = 4          # heads per core
HD = 128         # head dim
RD = 64          # rope / content half-width
ROPE_BASE = 10000.0
SCALE = float(HD) ** -0.5

SB = 512         # s-block for projections / q-blocks in attention
KB = 128         # k-block in attention
NKC = H // 128   # 16 contraction chunks over H
NLC = L // 128   # 2 contraction chunks over L

# Matmul input dtype: "f32" (exact, 4 cyc/row) or "f32r" (1 cyc/row, ~tf32).
MM_DTYPE = os.environ.get("MLA_MM_DTYPE", "f32")


def build_nc(s=S, mm_dtype=MM_DTYPE):
    """Build the Bass module for one core. `s` can be shrunk (multiple of 512)
    for simulator testing."""
    from concourse import bacc
    import concourse.bass as bass
    import concourse.mybir as mybir
    import concourse.tile as tile
    from concourse.masks import make_identity
    from contextlib import ExitStack

    f32 = mybir.dt.float32
    f32r = mybir.dt.float32r

    def mm(ap):
        # matmul-operand view; bitcast is free (same element size)
        return ap.bitcast(f32r) if mm_dtype == "f32r" else ap

    nsb = s // SB        # s-blocks
    nsc = s // 128       # 128-row s-chunks

    nc = bacc.Bacc(None, target_bir_lowering=False)

    hs = nc.dram_tensor("hs", [s, H], f32, kind="ExternalInput")
    w_down = nc.dram_tensor("w_down", [H, 512], f32, kind="ExternalInput")
    w_rk = nc.dram_tensor("w_rk", [H, HPC * RD], f32, kind="ExternalInput")
    w_qcomb = nc.dram_tensor("w_qcomb", [L, HPC * HD], f32, kind="ExternalInput")
    w_ku = nc.dram_tensor("w_ku", [L, HPC * RD], f32, kind="ExternalInput")
    w_vu = nc.dram_tensor("w_vu", [L, HPC * HD], f32, kind="ExternalInput")
    w_o = nc.dram_tensor("w_o", [HPC * HD, H], f32, kind="ExternalInput")
    # cos/sin halves replicated across all four 32-partition quadrants so any
    # 32-row operand can pair with a table slice at the SAME base partition
    # (walrus: both-SBUF tensor_tensor inputs must share base partition).
    rope_cc = nc.dram_tensor("rope_cc", [128, s], f32, kind="ExternalInput")
    rope_ss = nc.dram_tensor("rope_ss", [128, s], f32, kind="ExternalInput")
    out = nc.dram_tensor("out", [s, H], f32, kind="ExternalOutput")

    Exp = mybir.ActivationFunctionType.Exp
    is_ge = mybir.AluOpType.is_ge

    with ExitStack() as top:
        tc = top.enter_context(tile.TileContext(nc))

        # ---- persistent small pools -------------------------------------
        const_pool = top.enter_context(tc.tile_pool(name="const", bufs=1))
        ident = const_pool.tile([128, 128], f32, tag="ident")
        make_identity(nc, ident)
        ones128 = const_pool.tile([128, 128], f32, tag="ones")
        nc.gpsimd.memset(ones128[:], 1.0)
        cc_t = const_pool.tile([128, s], f32, tag="ropec")
        nc.sync.dma_start(cc_t[:], rope_cc[:])
        ss_t = const_pool.tile([128, s], f32, tag="ropes")
        nc.sync.dma_start(ss_t[:], rope_ss[:])

        wsmall = top.enter_context(tc.tile_pool(name="wsmall", bufs=1))
        w_qcomb_t = wsmall.tile([128, NLC, HPC * HD], f32, tag="wqc")
        nc.sync.dma_start(
            w_qcomb_t[:], w_qcomb.rearrange("(ko p) m -> p ko m", p=128))
        w_ku_t = wsmall.tile([128, NLC, HPC * RD], f32, tag="wku")
        nc.sync.dma_start(
            w_ku_t[:], w_ku.rearrange("(ko p) m -> p ko m", p=128))
        w_vu_t = wsmall.tile([128, NLC, HPC * HD], f32, tag="wvu")
        nc.sync.dma_start(
            w_vu_t[:], w_vu.rearrange("(ko p) m -> p ko m", p=128))

        # ---- latent / rope-k tensors (persist through attention) --------
        lat_pool = top.enter_context(tc.tile_pool(name="lat", bufs=1))
        kv_dT = lat_pool.tile([128, NLC, s], f32, tag="kvd")   # [L, s]
        q_dT = lat_pool.tile([128, NLC, s], f32, tag="qd")     # [L, s]
        k_rT = lat_pool.tile([128, NLC, s], f32, tag="krt")    # [4h*64, s]

        # ================= PHASE A: transpose hs + down/rope projections =
        with ExitStack() as pa:
            srcp = pa.enter_context(tc.tile_pool(name="src", bufs=4))
            hstp = pa.enter_context(tc.tile_pool(name="hst", bufs=1))
            wdp = pa.enter_context(tc.tile_pool(name="wdown", bufs=1))
            wrkp = pa.enter_context(tc.tile_pool(name="wrk", bufs=1))
            pst = pa.enter_context(tc.tile_pool(name="pst", bufs=2, space="PSUM"))
            psa = pa.enter_context(tc.tile_pool(name="psa", bufs=1, space="PSUM"))

            w_down_t = wdp.tile([128, NKC, 512], f32, tag="wd")
            nc.sync.dma_start(
                w_down_t[:], w_down.rearrange("(ko p) m -> p ko m", p=128))
            w_rk_t = wrkp.tile([128, NKC, HPC * RD], f32, tag="wr")
            nc.sync.dma_start(
                w_rk_t[:], w_rk.rearrange("(ko p) m -> p ko m", p=128))

            names = ("kv0", "kv1", "q0", "q1", "kr0", "kr1")
            for sb in range(nsb):
                hsT = hstp.tile([128, NKC, SB], f32, tag="hsT")
                srcs = []
                for sc in range(4):
                    srct = srcp.tile([128, H], f32, tag="src")
                    nc.sync.dma_start(
                        srct[:],
                        hs[sb * SB + sc * 128: sb * SB + (sc + 1) * 128, :])
                    srcs.append(srct)

                # 6 accumulation banks for this s-block
                pb = {n: psa.tile([128, SB], f32, tag=f"psa_{n}",
                                  name=f"psa_{n}_{sb}") for n in names}

                for kc in range(NKC):
                    pt = pst.tile([128, SB], f32, tag="pt")
                    for sc in range(4):
                        nc.tensor.transpose(
                            pt[:, sc * 128:(sc + 1) * 128],
                            srcs[sc][:, kc * 128:(kc + 1) * 128],
                            ident[:])
                    nc.any.tensor_copy(hsT[:, kc, :], pt[:])
                    rhs = mm(hsT[:, kc, :])
                    st = kc == 0
                    sp = kc == NKC - 1
                    nc.tensor.matmul(pb["kv0"][:], mm(w_down_t[:, kc, 0:128]),
                                     rhs, start=st, stop=sp)
                    nc.tensor.matmul(pb["kv1"][:], mm(w_down_t[:, kc, 128:256]),
                                     rhs, start=st, stop=sp)
                    nc.tensor.matmul(pb["q0"][:], mm(w_down_t[:, kc, 256:384]),
                                     rhs, start=st, stop=sp)
                    nc.tensor.matmul(pb["q1"][:], mm(w_down_t[:, kc, 384:512]),
                                     rhs, start=st, stop=sp)
                    nc.tensor.matmul(pb["kr0"][:], mm(w_rk_t[:, kc, 0:128]),
                                     rhs, start=st, stop=sp)
                    nc.tensor.matmul(pb["kr1"][:], mm(w_rk_t[:, kc, 128:256]),
                                     rhs, start=st, stop=sp)

                sbs = slice(sb * SB, (sb + 1) * SB)
                nc.any.tensor_copy(kv_dT[:, 0, sbs], pb["kv0"][:])
                nc.any.tensor_copy(kv_dT[:, 1, sbs], pb["kv1"][:])
                nc.any.tensor_copy(q_dT[:, 0, sbs], pb["q0"][:])
                nc.any.tensor_copy(q_dT[:, 1, sbs], pb["q1"][:])
                nc.any.tensor_copy(k_rT[:, 0, sbs], pb["kr0"][:])
                nc.any.tensor_copy(k_rT[:, 1, sbs], pb["kr1"][:])

        # ================= PHASES B-C: v, per-head q/k + attention =======
        # v_all / yT_all outlive the B/C transients (phase D reads yT_all),
        # so their pools go on the outermost stack, created after phase A's
        # pools have been closed.
        vp = top.enter_context(tc.tile_pool(name="vp", bufs=1))
        yp = top.enter_context(tc.tile_pool(name="yp", bufs=1))
        v_all = vp.tile([128, nsc, HPC * HD], f32, tag="vall")
        yT_all = yp.tile([128, HPC, s], f32, tag="yT")

        with ExitStack() as pbc:
            qkp = pbc.enter_context(tc.tile_pool(name="qkp", bufs=2))
            esp = pbc.enter_context(tc.tile_pool(name="esp", bufs=3))
            accp = pbc.enter_context(tc.tile_pool(name="accp", bufs=2))
            recp = pbc.enter_context(tc.tile_pool(name="recp", bufs=2))
            rtmp = pbc.enter_context(tc.tile_pool(name="rtmp", bufs=1))
            ps_qk = pbc.enter_context(
                tc.tile_pool(name="ps_qk", bufs=2, space="PSUM"))
            ps_s = pbc.enter_context(
                tc.tile_pool(name="ps_s", bufs=2, space="PSUM"))
            ps_y = pbc.enter_context(
                tc.tile_pool(name="ps_y", bufs=1, space="PSUM"))
            ps_b = pbc.enter_context(
                tc.tile_pool(name="ps_b", bufs=1, space="PSUM"))
            ps_v = pbc.enter_context(
                tc.tile_pool(name="ps_v", bufs=2, space="PSUM"))

            # ---- PHASE B: v (natural layout), all 4 heads ----
            for sc in range(nsc):
                pv = ps_v.tile([128, HPC * HD], f32, tag="pv")
                for lc in range(NLC):
                    nc.tensor.matmul(
                        pv[:], mm(kv_dT[:, lc, sc * 128:(sc + 1) * 128]),
                        mm(w_vu_t[:, lc, :]),
                        start=(lc == 0), stop=(lc == NLC - 1))
                nc.any.tensor_copy(v_all[:, sc, :], pv[:])

            def rope_into(dst_tile, dst_col, lo_src, hi_src, lo_base, sb):
                """dst[64:128, cols] = rope(src rows lo/hi) for one s-block.
                lo_src/hi_src are [32, SB] APs at partition bases lo_base and
                lo_base+32; table slices are taken at the matching bases."""
                sbs = slice(sb * SB, (sb + 1) * SB)
                lob = slice(lo_base, lo_base + 32)
                hib = slice(lo_base + 32, lo_base + 64)
                t1 = rtmp.tile([32, SB], f32, tag="t1")
                t2 = rtmp.tile([32, SB], f32, tag="t2")
                t3 = rtmp.tile([32, SB], f32, tag="t3")
                t4 = rtmp.tile([32, SB], f32, tag="t4")
                nc.vector.tensor_mul(t1[:], lo_src, cc_t[lob, sbs])
                nc.vector.tensor_mul(t2[:], hi_src, ss_t[hib, sbs])
                nc.vector.tensor_sub(dst_tile[64:96, dst_col], t1[:], t2[:])
                nc.vector.tensor_mul(t3[:], hi_src, cc_t[hib, sbs])
                nc.vector.tensor_mul(t4[:], lo_src, ss_t[lob, sbs])
                nc.vector.tensor_add(dst_tile[96:128, dst_col], t3[:], t4[:])

            for hp in range(2):            # head pairs
                h0, h1 = 2 * hp, 2 * hp + 1
                qT = {h: qkp.tile([128, s], f32, tag="qT", name=f"qT_{h}")
                      for h in (h0, h1)}
                kT = {h: qkp.tile([128, s], f32, tag="kT", name=f"kT_{h}")
                      for h in (h0, h1)}

                # ---- C1: projections + rope for both heads of the pair --
                for sb in range(nsb):
                    sbs = slice(sb * SB, (sb + 1) * SB)
                    # k content for both heads in one [128, SB] psum
                    pk = ps_qk.tile([128, SB], f32, tag="pqk")
                    for lc in range(NLC):
                        nc.tensor.matmul(
                            pk[:],
                            mm(w_ku_t[:, lc, hp * 128:(hp + 1) * 128]),
                            mm(kv_dT[:, lc, sbs]),
                            start=(lc == 0), stop=(lc == NLC - 1))
                    # evict: head h0 content -> kT[h0][0:64]
                    nc.vector.tensor_copy(kT[h0][0:64, sbs], pk[0:64, :])
                    # head h1 content rows 64:128 -> kT[h1][0:64] (two
                    # quadrant-aligned 32-partition moves)
                    nc.vector.tensor_copy(kT[h1][0:32, sbs], pk[64:96, :])
                    nc.vector.tensor_copy(kT[h1][32:64, sbs], pk[96:128, :])

                    for h in (h0, h1):
                        # q: content + rope pre-rotation in one psum
                        pq = ps_qk.tile([128, SB], f32, tag="pqk")
                        for lc in range(NLC):
                            nc.tensor.matmul(
                                pq[:],
                                mm(w_qcomb_t[:, lc, h * 128:(h + 1) * 128]),
                                mm(q_dT[:, lc, sbs]),
                                start=(lc == 0), stop=(lc == NLC - 1))
                        nc.vector.tensor_copy(qT[h][0:64, sbs], pq[0:64, :])
                        rope_into(qT[h], sbs, pq[64:96, :], pq[96:128, :],
                                  64, sb)
                        # k rope from k_rT rows for head h
                        base = (h % 2) * 64
                        lo = k_rT[base:base + 32, h // 2, sbs]
                        hi = k_rT[base + 32:base + 64, h // 2, sbs]
                        rope_into(kT[h], sbs, lo, hi, base, sb)

                # ---- C2: causal attention per head ----------------------
                for h in (h0, h1):
                    nqb = s // SB
                    for qi in range(nqb):
                        nkj = (qi + 1) * (SB // KB)       # k-blocks kept
                        qs = slice(qi * SB, (qi + 1) * SB)
                        acc = accp.tile([128, SB], f32, tag="acc")
                        py = ps_y.tile([128, SB], f32, tag="py")

                        def score_block(kj):
                            ps = ps_s.tile([128, SB], f32, tag="ps")
                            nc.tensor.matmul(
                                ps[:],
                                mm(kT[h][:, kj * KB:(kj + 1) * KB]),
                                mm(qT[h][:, qs]),
                                start=True, stop=True)
                            return ps

                        def finish_block(kj, ps):
                            es = esp.tile([128, SB], f32, tag="es")
                            nc.scalar.activation(es[:], ps[:], Exp, scale=SCALE)
                            if kj >= qi * (SB // KB):     # diagonal block
                                nc.gpsimd.affine_select(
                                    out=es[:], in_=es[:],
                                    compare_op=is_ge, fill=0.0,
                                    base=qi * SB - kj * KB,
                                    pattern=[[1, SB]],
                                    channel_multiplier=-1)
                            if kj == 0:
                                nc.vector.tensor_copy(acc[:], es[:])
                            else:
                                nc.vector.tensor_add(acc[:], acc[:], es[:])
                            nc.tensor.matmul(
                                py[:],
                                mm(v_all[:, kj, h * HD:(h + 1) * HD]),
                                mm(es[:]),
                                start=(kj == 0), stop=(kj == nkj - 1))

                        prev = score_block(0)
                        for kj in range(1, nkj):
                            cur = score_block(kj)
                            finish_block(kj - 1, prev)
                            prev = cur
                        finish_block(nkj - 1, prev)

                        # denominator: sum over all k partitions, broadcast
                        pbc_t = ps_b.tile([128, SB], f32, tag="pb")
                        nc.tensor.matmul(pbc_t[:], mm(ones128[:]), mm(acc[:]),
                                         start=True, stop=True)
                        rec = recp.tile([128, SB], f32, tag="rec")
                        nc.vector.reciprocal(rec[:], pbc_t[:])
                        nc.vector.tensor_mul(yT_all[:, h, qs], py[:], rec[:])

        # ================= PHASE D: o-projection =========================
        with ExitStack() as pd:
            wop = pd.enter_context(tc.tile_pool(name="wop", bufs=2))
            outp = pd.enter_context(tc.tile_pool(name="outp", bufs=4))
            ps_o = pd.enter_context(
                tc.tile_pool(name="ps_o", bufs=2, space="PSUM"))

            for ncol in range(H // 512):
                wo_t = wop.tile([128, HPC, 512], f32, tag="wo")
                nc.sync.dma_start(
                    wo_t[:],
                    w_o[:, ncol * 512:(ncol + 1) * 512].rearrange(
                        "(ho p) m -> p ho m", p=128))
                for sc in range(nsc):
                    po = ps_o.tile([128, 512], f32, tag="po")
                    for hh in range(HPC):
                        nc.tensor.matmul(
                            po[:],
                            mm(yT_all[:, hh, sc * 128:(sc + 1) * 128]),
                            mm(wo_t[:, hh, :]),
                            start=(hh == 0), stop=(hh == HPC - 1))
                    ot = outp.tile([128, 512], f32, tag="ot")
                    nc.any.tensor_copy(ot[:], po[:])
                    nc.sync.dma_start(
                        out[sc * 128:(sc + 1) * 128,
                            ncol * 512:(ncol + 1) * 512], ot[:])

    nc.compile()
    return nc


# ======================= host-side preparation ==========================

def _rope_tables(s):
    inv_freq = 1.0 / (ROPE_BASE ** (np.arange(0, RD, 2, dtype=np.float64) / RD))
    t = np.arange(s, dtype=np.float64)
    freqs = np.outer(t, inv_freq)                    # [s, 32]
    cc = np.tile(np.cos(freqs).T, (4, 1)).astype(np.float32)   # [128, s]
    ss = np.tile(np.sin(freqs).T, (4, 1)).astype(np.float32)
    return np.ascontiguousarray(cc), np.ascontiguousarray(ss)


def make_in_maps(hidden_states, Wkv_d, Wq_d, Wk_u, Wq_u, Wv_u, Wrk, Wrq, Wo,
                 s=S):
    f32 = np.float32
    w_down = np.ascontiguousarray(
        np.concatenate([Wkv_d, Wq_d], axis=1), dtype=f32)       # [H, 512]
    rope_cc, rope_ss = _rope_tables(s)
    Wk_u4 = Wk_u.reshape(L, NH, RD)
    Wq_u4 = Wq_u.reshape(L, NH, RD)
    Wrq4 = Wrq.reshape(L, NH, RD)
    Wv_u4 = Wv_u.reshape(L, NH, HD)
    Wrk4 = Wrk.reshape(H, NH, RD)
    Wo4 = Wo.reshape(NH, HD, H)

    in_maps = []
    for c in range(8):
        b, g = divmod(c, 4)
        hsel = slice(g * HPC, (g + 1) * HPC)
        qcomb = np.concatenate(
            [Wq_u4[:, hsel, :], Wrq4[:, hsel, :]], axis=2)      # [L, 4, 128]
        in_maps.append({
            "hs": np.ascontiguousarray(hidden_states[b, :s], dtype=f32),
            "w_down": w_down,
            "w_rk": np.ascontiguousarray(
                Wrk4[:, hsel, :].reshape(H, HPC * RD), dtype=f32),
            "w_qcomb": np.ascontiguousarray(
                qcomb.reshape(L, HPC * HD), dtype=f32),
            "w_ku": np.ascontiguousarray(
                Wk_u4[:, hsel, :].reshape(L, HPC * RD), dtype=f32),
            "w_vu": np.ascontiguousarray(
                Wv_u4[:, hsel, :].reshape(L, HPC * HD), dtype=f32),
            "w_o": np.ascontiguousarray(
                Wo4[hsel].reshape(HPC * HD, H), dtype=f32),
            "rope_cc": rope_cc,
            "rope_ss": rope_ss,
        })
    return in_maps


_NC_CACHE = {}


def kernel(hidden_states, Wkv_d, Wq_d, Wk_u, Wq_u, Wv_u, Wrk, Wrq, Wo):
    from concourse.bass_utils import run_bass_kernel_spmd

    key = (S, MM_DTYPE)
    if key not in _NC_CACHE:
        _NC_CACHE[key] = build_nc(S, MM_DTYPE)
    nc = _NC_CACHE[key]

    in_maps = make_in_maps(
        np.asarray(hidden_states), np.asarray(Wkv_d), np.asarray(Wq_d),
        np.asarray(Wk_u), np.asarray(Wq_u), np.asarray(Wv_u),
        np.asarray(Wrk), np.asarray(Wrq), np.asarray(Wo))

    res = run_bass_kernel_spmd(nc, in_maps, core_ids=list(range(8)))
    parts = [r["out"] for r in res.results]
    out = np.empty((B, S, H), dtype=np.float32)
    for b in range(B):
        out[b] = parts[4 * b] + parts[4 * b + 1] + parts[4 * b + 2] + parts[4 * b + 3]
    return out


# revision 23
# speedup vs baseline: 1.7413x; 1.7413x over previous
"""Trainium2 Bass kernel for MultiHeadLatentAttention (MLA), 8-core SPMD.

Sharding: core c = (batch b=c//4, head-group g=c%4 of 4 heads).
Each core computes the full latent down-projections for its batch
(replicated across the 4 cores of that batch), head-sharded
up-projections + RoPE + causal attention, and a partial o-projection
(its heads' rows of Wo). The host sums the 4 partial outputs per batch.

Shapes (fixed): B=2, S=2048, H=2048, L=256, nh=16, hd=128, rd=64.

Layouts on device (all transposed, feature-on-partitions):
  kv_dT, q_dT [L=256 -> 2x128, s]   k_rT [4 heads * 64 -> 2x128, s]
  qT_h, kT_h  [128 (64 content + 64 rope), s]  per head
  v_all       [128 (s%128), s//128, 4 heads * 128]  (natural v)
  yT_all      [128 (hd), 4 heads, s]

Attention is computed in scores-transposed orientation S^T[k, q] so the
probabilities feed the AV matmul directly (lhsT = v block, rhs = expS).
Softmax skips the max-subtraction (scores are provably tiny here:
|s| < ~2) and gets the denominator from an all-ones matmul over the
accumulated exp blocks, which also broadcasts it across partitions.
"""

import sys
import os

sys.path.insert(0, "/opt/trn_rl_repo")

import numpy as np

B = 2
S = 2048
H = 2048
L = 256          # latent dim (2 chunks of 128)
NH = 16          # total heads
HPC = 4          # heads per core
HD = 128         # head dim
RD = 64          # rope / content half-width
ROPE_BASE = 10000.0
SCALE = float(HD) ** -0.5

SB = 512         # s-block for projections / q-blocks in attention
KB = 128         # k-block in attention
NKC = H // 128   # 16 contraction chunks over H
NLC = L // 128   # 2 contraction chunks over L

# Matmul input dtype: "f32" (exact, 4 cyc/row) or "f32r" (1 cyc/row, ~tf32).
MM_DTYPE = os.environ.get("MLA_MM_DTYPE", "f32")


def build_nc(s=S, mm_dtype=MM_DTYPE):
    """Build the Bass module for one core. `s` can be shrunk (multiple of 512)
    for simulator testing."""
    from concourse import bacc
    import concourse.bass as bass
    import concourse.mybir as mybir
    import concourse.tile as tile
    from concourse.masks import make_identity
    from contextlib import ExitStack

    f32 = mybir.dt.float32
    f32r = mybir.dt.float32r

    # dtype for every tile/DRAM tensor that feeds a matmul: the BIR
    # verifier requires producers of fp32r-matmul operands to WRITE fp32r.
    mdt = f32r if mm_dtype == "f32r" else f32

    nsb = s // SB        # s-blocks
    nsc = s // 128       # 128-row s-chunks

    nc = bacc.Bacc(None, target_bir_lowering=False)

    hs = nc.dram_tensor("hs", [s, H], f32, kind="ExternalInput")
    w_down = nc.dram_tensor("w_down", [H, 512], mdt, kind="ExternalInput")
    w_rk = nc.dram_tensor("w_rk", [H, HPC * RD], mdt, kind="ExternalInput")
    w_qcomb = nc.dram_tensor("w_qcomb", [L, HPC * HD], mdt, kind="ExternalInput")
    w_ku = nc.dram_tensor("w_ku", [L, HPC * RD], mdt, kind="ExternalInput")
    w_vu = nc.dram_tensor("w_vu", [L, HPC * HD], mdt, kind="ExternalInput")
    w_o = nc.dram_tensor("w_o", [HPC * HD, H], mdt, kind="ExternalInput")
    # cos/sin halves replicated across all four 32-partition quadrants so any
    # 32-row operand can pair with a table slice at the SAME base partition
    # (walrus: both-SBUF tensor_tensor inputs must share base partition).
    rope_cc = nc.dram_tensor("rope_cc", [128, s], f32, kind="ExternalInput")
    rope_ss = nc.dram_tensor("rope_ss", [128, s], f32, kind="ExternalInput")
    out = nc.dram_tensor("out", [s, H], f32, kind="ExternalOutput")

    Exp = mybir.ActivationFunctionType.Exp
    is_ge = mybir.AluOpType.is_ge

    with ExitStack() as top:
        tc = top.enter_context(tile.TileContext(nc))

        # ---- persistent small pools -------------------------------------
        const_pool = top.enter_context(tc.tile_pool(name="const", bufs=1))
        ident = const_pool.tile([128, 128], f32, tag="ident")
        make_identity(nc, ident)
        ones128 = const_pool.tile([128, 128], mdt, tag="ones")
        if mdt == f32:
            nc.gpsimd.memset(ones128[:], 1.0)
        else:
            # memset can't write f32r; stage in f32 and convert via copy
            ones_f32 = const_pool.tile([128, 128], f32, tag="ones_f32")
            nc.gpsimd.memset(ones_f32[:], 1.0)
            nc.vector.tensor_copy(ones128[:], ones_f32[:])
        cc_t = const_pool.tile([128, s], f32, tag="ropec")
        nc.sync.dma_start(cc_t[:], rope_cc[:])
        ss_t = const_pool.tile([128, s], f32, tag="ropes")
        nc.sync.dma_start(ss_t[:], rope_ss[:])

        wsmall = top.enter_context(tc.tile_pool(name="wsmall", bufs=1))
        w_qcomb_t = wsmall.tile([128, NLC, HPC * HD], mdt, tag="wqc")
        nc.sync.dma_start(
            w_qcomb_t[:], w_qcomb.rearrange("(ko p) m -> p ko m", p=128))
        w_ku_t = wsmall.tile([128, NLC, HPC * RD], mdt, tag="wku")
        nc.sync.dma_start(
            w_ku_t[:], w_ku.rearrange("(ko p) m -> p ko m", p=128))
        w_vu_t = wsmall.tile([128, NLC, HPC * HD], mdt, tag="wvu")
        nc.sync.dma_start(
            w_vu_t[:], w_vu.rearrange("(ko p) m -> p ko m", p=128))

        # ---- latent / rope-k tensors (persist through attention) --------
        lat_pool = top.enter_context(tc.tile_pool(name="lat", bufs=1))
        kv_dT = lat_pool.tile([128, NLC, s], mdt, tag="kvd")   # [L, s]
        q_dT = lat_pool.tile([128, NLC, s], mdt, tag="qd")     # [L, s]
        k_rT = lat_pool.tile([128, NLC, s], mdt, tag="krt")    # [4h*64, s]

        # ================= PHASE A: transpose hs + down/rope projections =
        with ExitStack() as pa:
            srcp = pa.enter_context(tc.tile_pool(name="src", bufs=4))
            hstp = pa.enter_context(tc.tile_pool(name="hst", bufs=1))
            wdp = pa.enter_context(tc.tile_pool(name="wdown", bufs=1))
            wrkp = pa.enter_context(tc.tile_pool(name="wrk", bufs=1))
            pst = pa.enter_context(tc.tile_pool(name="pst", bufs=2, space="PSUM"))
            psa = pa.enter_context(tc.tile_pool(name="psa", bufs=1, space="PSUM"))

            w_down_t = wdp.tile([128, NKC, 512], mdt, tag="wd")
            nc.sync.dma_start(
                w_down_t[:], w_down.rearrange("(ko p) m -> p ko m", p=128))
            w_rk_t = wrkp.tile([128, NKC, HPC * RD], mdt, tag="wr")
            nc.sync.dma_start(
                w_rk_t[:], w_rk.rearrange("(ko p) m -> p ko m", p=128))

            names = ("kv0", "kv1", "q0", "q1", "kr0", "kr1")
            for sb in range(nsb):
                hsT = hstp.tile([128, NKC, SB], mdt, tag="hsT")
                srcs = []
                for sc in range(4):
                    srct = srcp.tile([128, H], f32, tag="src")
                    nc.sync.dma_start(
                        srct[:],
                        hs[sb * SB + sc * 128: sb * SB + (sc + 1) * 128, :])
                    srcs.append(srct)

                # 6 accumulation banks for this s-block
                pb = {n: psa.tile([128, SB], f32, tag=f"psa_{n}",
                                  name=f"psa_{n}_{sb}") for n in names}

                for kc in range(NKC):
                    pt = pst.tile([128, SB], f32, tag="pt")
                    for sc in range(4):
                        nc.tensor.transpose(
                            pt[:, sc * 128:(sc + 1) * 128],
                            srcs[sc][:, kc * 128:(kc + 1) * 128],
                            ident[:])
                    nc.any.tensor_copy(hsT[:, kc, :], pt[:])
                    rhs = (hsT[:, kc, :])
                    st = kc == 0
                    sp = kc == NKC - 1
                    nc.tensor.matmul(pb["kv0"][:], (w_down_t[:, kc, 0:128]),
                                     rhs, start=st, stop=sp)
                    nc.tensor.matmul(pb["kv1"][:], (w_down_t[:, kc, 128:256]),
                                     rhs, start=st, stop=sp)
                    nc.tensor.matmul(pb["q0"][:], (w_down_t[:, kc, 256:384]),
                                     rhs, start=st, stop=sp)
                    nc.tensor.matmul(pb["q1"][:], (w_down_t[:, kc, 384:512]),
                                     rhs, start=st, stop=sp)
                    nc.tensor.matmul(pb["kr0"][:], (w_rk_t[:, kc, 0:128]),
                                     rhs, start=st, stop=sp)
                    nc.tensor.matmul(pb["kr1"][:], (w_rk_t[:, kc, 128:256]),
                                     rhs, start=st, stop=sp)

                sbs = slice(sb * SB, (sb + 1) * SB)
                nc.any.tensor_copy(kv_dT[:, 0, sbs], pb["kv0"][:])
                nc.any.tensor_copy(kv_dT[:, 1, sbs], pb["kv1"][:])
                nc.any.tensor_copy(q_dT[:, 0, sbs], pb["q0"][:])
                nc.any.tensor_copy(q_dT[:, 1, sbs], pb["q1"][:])
                nc.any.tensor_copy(k_rT[:, 0, sbs], pb["kr0"][:])
                nc.any.tensor_copy(k_rT[:, 1, sbs], pb["kr1"][:])

        # ================= PHASES B-C: v, per-head q/k + attention =======
        # v_all / yT_all outlive the B/C transients (phase D reads yT_all),
        # so their pools go on the outermost stack, created after phase A's
        # pools have been closed.
        vp = top.enter_context(tc.tile_pool(name="vp", bufs=1))
        yp = top.enter_context(tc.tile_pool(name="yp", bufs=1))
        v_all = vp.tile([128, nsc, HPC * HD], mdt, tag="vall")
        yT_all = yp.tile([128, HPC, s], mdt, tag="yT")

        with ExitStack() as pbc:
            qkp = pbc.enter_context(tc.tile_pool(name="qkp", bufs=2))
            esp = pbc.enter_context(tc.tile_pool(name="esp", bufs=3))
            accp = pbc.enter_context(tc.tile_pool(name="accp", bufs=2))
            recp = pbc.enter_context(tc.tile_pool(name="recp", bufs=2))
            rtmp = pbc.enter_context(tc.tile_pool(name="rtmp", bufs=1))
            # ---- PHASE B: v (natural layout), all 4 heads ----
            # scoped psum pool, closed before phase C's psum pools open
            with tc.tile_pool(name="ps_v", bufs=2, space="PSUM") as ps_v:
                for sc in range(nsc):
                    pv = ps_v.tile([128, HPC * HD], f32, tag="pv")
                    for lc in range(NLC):
                        nc.tensor.matmul(
                            pv[:], (kv_dT[:, lc, sc * 128:(sc + 1) * 128]),
                            (w_vu_t[:, lc, :]),
                            start=(lc == 0), stop=(lc == NLC - 1))
                    nc.any.tensor_copy(v_all[:, sc, :], pv[:])

            ps_qk = pbc.enter_context(
                tc.tile_pool(name="ps_qk", bufs=2, space="PSUM"))
            ps_s = pbc.enter_context(
                tc.tile_pool(name="ps_s", bufs=2, space="PSUM"))
            ps_y = pbc.enter_context(
                tc.tile_pool(name="ps_y", bufs=2, space="PSUM"))
            ps_b = pbc.enter_context(
                tc.tile_pool(name="ps_b", bufs=2, space="PSUM"))

            def rope_into(dst_tile, dst_col, lo_src, hi_src, lo_base, sb):
                """dst[64:128, cols] = rope(src rows lo/hi) for one s-block.
                lo_src/hi_src are [32, SB] APs at partition bases lo_base and
                lo_base+32; table slices are taken at the matching bases."""
                sbs = slice(sb * SB, (sb + 1) * SB)
                lob = slice(lo_base, lo_base + 32)
                hib = slice(lo_base + 32, lo_base + 64)
                t1 = rtmp.tile([32, SB], f32, tag="t1")
                t2 = rtmp.tile([32, SB], f32, tag="t2")
                t3 = rtmp.tile([32, SB], f32, tag="t3")
                t4 = rtmp.tile([32, SB], f32, tag="t4")
                nc.vector.tensor_mul(t1[:], lo_src, cc_t[lob, sbs])
                nc.vector.tensor_mul(t2[:], hi_src, ss_t[hib, sbs])
                nc.vector.tensor_sub(dst_tile[64:96, dst_col], t1[:], t2[:])
                nc.vector.tensor_mul(t3[:], hi_src, cc_t[hib, sbs])
                nc.vector.tensor_mul(t4[:], lo_src, ss_t[lob, sbs])
                nc.vector.tensor_add(dst_tile[96:128, dst_col], t3[:], t4[:])

            for hp in range(2):            # head pairs
                h0, h1 = 2 * hp, 2 * hp + 1
                qT = {h: qkp.tile([128, s], mdt, tag="qT", name=f"qT_{h}")
                      for h in (h0, h1)}
                kT = {h: qkp.tile([128, s], mdt, tag="kT", name=f"kT_{h}")
                      for h in (h0, h1)}

                # ---- C1: projections + rope for both heads of the pair --
                for sb in range(nsb):
                    sbs = slice(sb * SB, (sb + 1) * SB)
                    # k content for both heads in one [128, SB] psum
                    pk = ps_qk.tile([128, SB], f32, tag="pqk")
                    for lc in range(NLC):
                        nc.tensor.matmul(
                            pk[:],
                            (w_ku_t[:, lc, hp * 128:(hp + 1) * 128]),
                            (kv_dT[:, lc, sbs]),
                            start=(lc == 0), stop=(lc == NLC - 1))
                    # evict: head h0 content -> kT[h0][0:64]
                    nc.vector.tensor_copy(kT[h0][0:64, sbs], pk[0:64, :])
                    # head h1 content rows 64:128 -> kT[h1][0:64] (two
                    # quadrant-aligned 32-partition moves)
                    nc.vector.tensor_copy(kT[h1][0:32, sbs], pk[64:96, :])
                    nc.vector.tensor_copy(kT[h1][32:64, sbs], pk[96:128, :])

                    for h in (h0, h1):
                        # q: content + rope pre-rotation in one psum
                        pq = ps_qk.tile([128, SB], f32, tag="pqk")
                        for lc in range(NLC):
                            nc.tensor.matmul(
                                pq[:],
                                (w_qcomb_t[:, lc, h * 128:(h + 1) * 128]),
                                (q_dT[:, lc, sbs]),
                                start=(lc == 0), stop=(lc == NLC - 1))
                        nc.vector.tensor_copy(qT[h][0:64, sbs], pq[0:64, :])
                        rope_into(qT[h], sbs, pq[64:96, :], pq[96:128, :],
                                  64, sb)
                        # k rope from k_rT rows for head h
                        base = (h % 2) * 64
                        lo = k_rT[base:base + 32, h // 2, sbs].bitcast(f32)
                        hi = k_rT[base + 32:base + 64, h // 2, sbs].bitcast(f32)
                        rope_into(kT[h], sbs, lo, hi, base, sb)

                # ---- C2: causal attention per head ----------------------
                for h in (h0, h1):
                    nqb = s // SB
                    for qi in range(nqb):
                        nkj = (qi + 1) * (SB // KB)       # k-blocks kept
                        qs = slice(qi * SB, (qi + 1) * SB)
                        acc = accp.tile([128, SB], mdt, tag="acc")
                        py = ps_y.tile([128, SB], f32, tag="py")

                        def score_block(kj):
                            ps = ps_s.tile([128, SB], f32, tag="ps")
                            nc.tensor.matmul(
                                ps[:],
                                (kT[h][:, kj * KB:(kj + 1) * KB]),
                                (qT[h][:, qs]),
                                start=True, stop=True)
                            return ps

                        def finish_block(kj, ps):
                            es = esp.tile([128, SB], mdt, tag="es")
                            nc.scalar.activation(es[:], ps[:], Exp, scale=SCALE)
                            if kj >= qi * (SB // KB):     # diagonal block
                                nc.gpsimd.affine_select(
                                    out=es[:], in_=es[:],
                                    compare_op=is_ge, fill=0.0,
                                    base=qi * SB - kj * KB,
                                    pattern=[[1, SB]],
                                    channel_multiplier=-1)
                            if kj == 0:
                                nc.vector.tensor_copy(acc[:], es[:])
                            else:
                                nc.vector.tensor_add(acc[:], acc[:], es[:])
                            nc.tensor.matmul(
                                py[:],
                                (v_all[:, kj, h * HD:(h + 1) * HD]),
                                (es[:]),
                                start=(kj == 0), stop=(kj == nkj - 1))

                        prev = score_block(0)
                        for kj in range(1, nkj):
                            cur = score_block(kj)
                            finish_block(kj - 1, prev)
                            prev = cur
                        finish_block(nkj - 1, prev)

                        # denominator: sum over all k partitions, broadcast
                        pbc_t = ps_b.tile([128, SB], f32, tag="pb")
                        nc.tensor.matmul(pbc_t[:], (ones128[:]), (acc[:]),
                                         start=True, stop=True)
                        rec = recp.tile([128, SB], f32, tag="rec")
                        nc.vector.reciprocal(rec[:], pbc_t[:])
                        nc.vector.tensor_mul(yT_all[:, h, qs], py[:], rec[:])

        # ================= PHASE D: o-projection =========================
        with ExitStack() as pd:
            wop = pd.enter_context(tc.tile_pool(name="wop", bufs=2))
            outp = pd.enter_context(tc.tile_pool(name="outp", bufs=4))
            ps_o = pd.enter_context(
                tc.tile_pool(name="ps_o", bufs=2, space="PSUM"))

            for ncol in range(H // 512):
                wo_t = wop.tile([128, HPC, 512], mdt, tag="wo")
                nc.sync.dma_start(
                    wo_t[:],
                    w_o[:, ncol * 512:(ncol + 1) * 512].rearrange(
                        "(ho p) m -> p ho m", p=128))
                for sc in range(nsc):
                    po = ps_o.tile([128, 512], f32, tag="po")
                    for hh in range(HPC):
                        nc.tensor.matmul(
                            po[:],
                            (yT_all[:, hh, sc * 128:(sc + 1) * 128]),
                            (wo_t[:, hh, :]),
                            start=(hh == 0), stop=(hh == HPC - 1))
                    ot = outp.tile([128, 512], f32, tag="ot")
                    nc.any.tensor_copy(ot[:], po[:])
                    nc.sync.dma_start(
                        out[sc * 128:(sc + 1) * 128,
                            ncol * 512:(ncol + 1) * 512], ot[:])

    nc.compile()
    return nc


# ======================= host-side preparation ==========================

def _rope_tables(s):
    inv_freq = 1.0 / (ROPE_BASE ** (np.arange(0, RD, 2, dtype=np.float64) / RD))
    t = np.arange(s, dtype=np.float64)
    freqs = np.outer(t, inv_freq)                    # [s, 32]
    cc = np.tile(np.cos(freqs).T, (4, 1)).astype(np.float32)   # [128, s]
    ss = np.tile(np.sin(freqs).T, (4, 1)).astype(np.float32)
    return np.ascontiguousarray(cc), np.ascontiguousarray(ss)


def make_in_maps(hidden_states, Wkv_d, Wq_d, Wk_u, Wq_u, Wv_u, Wrk, Wrq, Wo,
                 s=S):
    f32 = np.float32
    w_down = np.ascontiguousarray(
        np.concatenate([Wkv_d, Wq_d], axis=1), dtype=f32)       # [H, 512]
    rope_cc, rope_ss = _rope_tables(s)
    Wk_u4 = Wk_u.reshape(L, NH, RD)
    Wq_u4 = Wq_u.reshape(L, NH, RD)
    Wrq4 = Wrq.reshape(L, NH, RD)
    Wv_u4 = Wv_u.reshape(L, NH, HD)
    Wrk4 = Wrk.reshape(H, NH, RD)
    Wo4 = Wo.reshape(NH, HD, H)

    in_maps = []
    for c in range(8):
        b, g = divmod(c, 4)
        hsel = slice(g * HPC, (g + 1) * HPC)
        qcomb = np.concatenate(
            [Wq_u4[:, hsel, :], Wrq4[:, hsel, :]], axis=2)      # [L, 4, 128]
        in_maps.append({
            "hs": np.ascontiguousarray(hidden_states[b, :s], dtype=f32),
            "w_down": w_down,
            "w_rk": np.ascontiguousarray(
                Wrk4[:, hsel, :].reshape(H, HPC * RD), dtype=f32),
            "w_qcomb": np.ascontiguousarray(
                qcomb.reshape(L, HPC * HD), dtype=f32),
            "w_ku": np.ascontiguousarray(
                Wk_u4[:, hsel, :].reshape(L, HPC * RD), dtype=f32),
            "w_vu": np.ascontiguousarray(
                Wv_u4[:, hsel, :].reshape(L, HPC * HD), dtype=f32),
            "w_o": np.ascontiguousarray(
                Wo4[hsel].reshape(HPC * HD, H), dtype=f32),
            "rope_cc": rope_cc,
            "rope_ss": rope_ss,
        })
    return in_maps


_NC_CACHE = {}


def kernel(hidden_states, Wkv_d, Wq_d, Wk_u, Wq_u, Wv_u, Wrk, Wrq, Wo):
    from concourse.bass_utils import run_bass_kernel_spmd

    key = (S, MM_DTYPE)
    if key not in _NC_CACHE:
        _NC_CACHE[key] = build_nc(S, MM_DTYPE)
    nc = _NC_CACHE[key]

    in_maps = make_in_maps(
        np.asarray(hidden_states), np.asarray(Wkv_d), np.asarray(Wq_d),
        np.asarray(Wk_u), np.asarray(Wq_u), np.asarray(Wv_u),
        np.asarray(Wrk), np.asarray(Wrq), np.asarray(Wo))

    res = run_bass_kernel_spmd(nc, in_maps, core_ids=list(range(8)))
    parts = [r["out"] for r in res.results]
    out = np.empty((B, S, H), dtype=np.float32)
    for b in range(B):
        out[b] = parts[4 * b] + parts[4 * b + 1] + parts[4 * b + 2] + parts[4 * b + 3]
    return out
